# revision 15
# baseline (speedup 1.0000x reference)
"""Trainium2 Bass kernel for the 14-wire quantum autoencoder swap test.

Math: wires 10-13 stay |0> until the swap test, so
P(aux=1) = 0.5 - 0.5*q with q = sum_{trash wires 7,8,9 = 0} |c_i|^2 of the
10-qubit state after AngleEmbedding + BasicEntanglerLayers.

All transforms run on the PE in fp16 (fp32 PSUM accumulation):
  state S~ [ft, w] per 16-sample half-batch, ft = par*64 + w0*32 + w1*16 + bl
  (par = w2 after C01,C12; w = wires 3..9 index, w9 = MSB).
  Per layer: stage G (RX0-2 + C01 + C12 [+ C90 of previous layer folded via
  row-permuted GkC90 blocks for the w9=1 column half]) flips to standard
  layout [w, ft']; stage K (RX3-9 + C34..C89, C23 via K2/K2b stationary split
  by par class) flips back. Zero-padded stationary columns keep every matmul
  writing all 128 PSUM partitions (fp32-family dst-partition ISA rule).
  Negations live in host-built blocks: [Xim_neg | Xre | Xim] per matrix.
Final: |.|^2 on w<16 cols, sel-matmul partition reduce, affine.
"""
import numpy as np

NCORES = 8
B_CORE = 32
HB = 16
DEPTH = 4
NQ = 10

# pk16 fp16 [128, 1088]: fsc 512 | pt 384 ([imn|re|im]) | zeros 192
C_FSC = 0          # per half: [Fre 128 | Fim 128] at rows hb*32+bl
C_PT = 512
C_ZERO = 896
PK16 = 1088

# pk32 f32 [128, 34]: cols 0:16 = -0.5*sel, col 16 = 1.0, cols 18:34 = 0.5
PK32 = 34

# mats fp16 [128, 5760]: per layer: gR 384 | gC 384 (l>=1) | kR 384 | kB 384
L_COLS = [1152, 1536, 1536, 1536]
L_OFF = [0, 1152, 2688, 4224]
M_COLS = 5760

# ---------------------------------------------------------------------------
# Host-side constant construction
# ---------------------------------------------------------------------------

# ft class (par*4 + gg) -> g = w0*4 + w1*2 + w2
_FT_G = np.zeros(8, dtype=np.int64)
for _par in range(2):
    for _gg in range(4):
        _FT_G[_par * 4 + _gg] = (_gg >> 1) * 4 + (_gg & 1) * 2 + _par


def _perm_matrix(perm):
    m = np.zeros((len(perm), len(perm)))
    for src, dst in enumerate(perm):
        m[dst, src] = 1.0
    return m


def _cnot_chain_perm_p():
    perm = np.zeros(128, dtype=np.int64)
    for p in range(128):
        w = [(p >> k) & 1 for k in range(7)]
        for k in range(6):
            w[k + 1] ^= w[k]
        perm[p] = sum(w[k] << k for k in range(7))
    return perm


def _build_k2(weights_l):
    m = np.array([[1.0]], dtype=np.complex128)
    for w in (9, 8, 7, 6, 5, 4, 3):
        c, s = np.cos(weights_l[w] / 2), np.sin(weights_l[w] / 2)
        r = np.array([[c, -1j * s], [-1j * s, c]], dtype=np.complex128)
        m = np.kron(m, r)
    qa = _perm_matrix(_cnot_chain_perm_p())
    k2 = qa @ m
    k2b = k2 @ _perm_matrix(np.arange(128) ^ 1)
    return k2, k2b


def _build_gk(weights_l):
    m = np.array([[1.0]], dtype=np.complex128)
    for w in (0, 1, 2):
        c, s = np.cos(weights_l[w] / 2), np.sin(weights_l[w] / 2)
        r = np.array([[c, -1j * s], [-1j * s, c]], dtype=np.complex128)
        m = np.kron(m, r)
    perm = np.zeros(8, dtype=np.int64)
    for g in range(8):
        w0, w1, w2 = (g >> 2) & 1, (g >> 1) & 1, g & 1
        w1 ^= w0
        w2 ^= w1
        perm[g] = w0 * 4 + w1 * 2 + w2
    G = _perm_matrix(perm) @ m
    gk = np.zeros((128, 128), dtype=np.complex128)
    for ci in range(8):
        for co in range(8):
            v = G[_FT_G[co], _FT_G[ci]]
            if v != 0:
                for bl in range(HB):
                    gk[ci * 16 + bl, co * 16 + bl] = v
    return gk  # contraction: T[ft_out] = sum_ft_in S[ft_in] * gk[ft_in, ft_out]


def _blocks(m):
    """[im_neg | re | im] fp column triple for complex matrix m [128,128]."""
    return np.concatenate([-m.imag, m.real, m.imag], axis=1)


def _make_mats(weights):
    wt = weights.astype(np.float64).reshape(DEPTH, NQ)
    flip = np.arange(128) ^ 32
    cols = []
    for l in range(DEPTH):
        gk = _build_gk(wt[l])
        k2, k2b = _build_k2(wt[l])
        cols.append(_blocks(gk))
        if l >= 1:
            cols.append(_blocks(gk[flip, :]))
        cols.append(_blocks(k2.T))
        cols.append(_blocks(k2b.T))
    mats = np.concatenate(cols, axis=1)
    assert mats.shape == (128, M_COLS)
    return mats.astype(np.float16)


def _make_pk16(features_core):
    th = features_core.astype(np.float64)
    B = th.shape[0]
    c_emb, s_emb = np.cos(th / 2), np.sin(th / 2)
    v = np.stack([c_emb.astype(np.complex128), -1j * s_emb], axis=-1)

    pk = np.zeros((128, PK16), dtype=np.float64)
    # rows: sample b = hb*16+bl lives at partition hb*32+bl
    rows = np.array([(b // HB) * 32 + (b % HB) for b in range(B)])
    # pt: seed over wires 3..9: col j = w9*64+...+w3 (w3 = LSB)
    for j in range(128):
        val = np.ones(B, dtype=np.complex128)
        for k, w in enumerate((3, 4, 5, 6, 7, 8, 9)):
            val = val * v[:, w, (j >> k) & 1]
        pk[rows, C_PT + j] = -val.imag
        pk[rows, C_PT + 128 + j] = val.real
        pk[rows, C_PT + 256 + j] = val.imag
    # fsc: per half: Fre/Fim [16, 128]; col ft = cls*16+bl, row hb*32+bl
    for hb in range(2):
        for cls in range(8):
            g = _FT_G[cls]
            w0, w1, w2 = (g >> 2) & 1, (g >> 1) & 1, g & 1
            for bl in range(HB):
                b = hb * HB + bl
                F = v[b, 0, w0] * v[b, 1, w1] * v[b, 2, w2]
                col = C_FSC + hb * 256
                pk[hb * 32 + bl, col + cls * 16 + bl] = F.real
                pk[hb * 32 + bl, col + 128 + cls * 16 + bl] = F.imag
    return pk.astype(np.float16)


def _make_pk32():
    pk = np.zeros((128, PK32), dtype=np.float32)
    for ft in range(128):
        pk[ft, ft % 16] = -0.5
    pk[:, 16] = 1.0
    pk[:, 18:34] = 0.5
    return pk


# ---------------------------------------------------------------------------
# Bass program
# ---------------------------------------------------------------------------

_PROGRAM = None


def _build_program():
    import concourse.bacc as bacc
    import concourse.mybir as mybir
    import concourse.tile as tile

    F32 = mybir.dt.float32
    F16 = mybir.dt.float16
    MULT = mybir.AluOpType.mult
    ADD = mybir.AluOpType.add

    nc = bacc.Bacc("TRN2", target_bir_lowering=False, debug=False,
                   num_devices=NCORES)

    d_pk16 = nc.dram_tensor("pk16", [128, PK16], F16, kind="ExternalInput")
    d_pk32 = nc.dram_tensor("pk32", [128, PK32], F32, kind="ExternalInput")
    d_mats = nc.dram_tensor("mats", [128, M_COLS], F16, kind="ExternalInput")
    d_out = nc.dram_tensor("out", [1, B_CORE], F32, kind="ExternalOutput")

    with tile.TileContext(nc) as tc:
        with (
            tc.tile_pool(name="const", bufs=1) as cpool,
            tc.tile_pool(name="psum", bufs=6, space="PSUM") as ppool,
            tc.tile_pool(name="psumb", bufs=2, space="PSUM") as bpool,
        ):
            t_pk16 = cpool.tile([128, PK16], F16, tag="pk16")
            t_pk32 = cpool.tile([128, PK32], F32, tag="pk32")
            t_mats = cpool.tile([128, M_COLS], F16, tag="mats")

            # DMAs in just-in-time order, single SP queue
            nc.sync.dma_start(t_pk16[0:64, 0:C_ZERO], d_pk16[0:64, 0:C_ZERO])
            nc.sync.dma_start(t_mats[:, 0:384], d_mats[:, 0:384])
            nc.sync.dma_start(t_pk16[:, C_ZERO:PK16], d_pk16[:, C_ZERO:PK16])
            nc.sync.dma_start(t_mats[:, 384:1152], d_mats[:, 384:1152])
            for l in range(1, DEPTH):
                sl = slice(L_OFF[l], L_OFF[l] + L_COLS[l])
                nc.sync.dma_start(t_mats[:, sl], d_mats[:, sl])
            nc.sync.dma_start(t_pk32[:], d_pk32[:])

            # PE warm-up: starts the PE ramp clock early
            t_wu = cpool.tile([128, 256], F16, tag="wu")
            nc.gpsimd.memset(t_wu[:], 0.0)
            ps_wu = ppool.tile([128, 128], F32, tag="ps", name="wu")
            for i in range(3):
                nc.tensor.matmul(ps_wu[:], t_wu[:, 0:128], t_wu[:, 0:128],
                                 start=True, stop=True)

            # state tiles (persistent; zero-padded layout [128, 512]:
            # data chunks at 0,128,256,384 (64 cols), Z at 64,192,320)
            def zfill(t):
                dst = t[:].rearrange("p (a b) -> p a b", a=4, b=128)
                src = t_pk16[:, C_ZERO:C_ZERO + 192].rearrange(
                    "p (a b) -> p a b", a=3, b=64)
                nc.gpsimd.tensor_copy(dst[:, 0:3, 64:128], src)

            s0 = []
            sAB = []
            tt = []
            for hb in range(2):
                s0.append(cpool.tile([128, 256], F16, tag=f"s0{hb}",
                                     name=f"s0{hb}"))
                a = cpool.tile([128, 512], F16, tag=f"sA{hb}", name=f"sA{hb}")
                b = cpool.tile([128, 512], F16, tag=f"sB{hb}", name=f"sB{hb}")
                t = cpool.tile([128, 512], F16, tag=f"tt{hb}", name=f"tt{hb}")
                sAB.append([a, b])
                tt.append(t)
            for hb in range(2):
                zfill(tt[hb])
            for hb in range(2):
                zfill(sAB[hb][0])
            for hb in range(2):
                zfill(sAB[hb][1])

            # chunk views of a zero-padded tile: [p, x(par/w9), y(ri), 64]
            def chunks(t):
                return t[:].rearrange("p (x y b) -> p x y b",
                                      x=2, y=2, b=128)[:, :, :, 0:64]

            # embedding matmuls (re/im split) -> psE -> s0
            psE = []
            ev = []
            for hb in range(2):
                r0 = hb * 32
                rows = slice(r0, r0 + HB)
                fre = t_pk16[r0:r0 + HB, C_FSC + hb * 256:C_FSC + hb * 256 + 128]
                fim = t_pk16[r0:r0 + HB,
                             C_FSC + hb * 256 + 128:C_FSC + hb * 256 + 256]
                ptimn = t_pk16[rows, C_PT:C_PT + 128]
                ptre = t_pk16[rows, C_PT + 128:C_PT + 256]
                ptim = t_pk16[rows, C_PT + 256:C_PT + 384]
                ev.append((fre, fim, ptimn, ptre, ptim))
                psEr = ppool.tile([128, 128], F32, tag="ps", name=f"per{hb}")
                psEi = ppool.tile([128, 128], F32, tag="ps", name=f"pei{hb}")
                psE.append((psEr, psEi))
                nc.tensor.matmul(psEr[:], fre, ptre, start=True, stop=False)
                nc.tensor.matmul(psEr[:], fim, ptimn, start=False, stop=True)
            for hb in range(2):
                nc.scalar.copy(s0[hb][:, 0:128], psE[hb][0][:])
            for hb in range(2):
                fre, fim, ptimn, ptre, ptim = ev[hb]
                nc.tensor.matmul(psE[hb][1][:], fre, ptim,
                                 start=True, stop=False)
                nc.tensor.matmul(psE[hb][1][:], fim, ptre,
                                 start=False, stop=True)
            for hb in range(2):
                nc.vector.tensor_copy(s0[hb][:, 128:256], psE[hb][1][:])

            # ---------------- layers ----------------
            def mat(c0, c1):
                return t_mats[:, c0:c1]

            psK3 = [None, None]

            def emit_g(l, hb, gR, gC):
                pgr = ppool.tile([128, 128], F32, tag="ps",
                                 name=f"pgr{l}{hb}")
                pgi = ppool.tile([128, 128], F32, tag="ps",
                                 name=f"pgi{l}{hb}")
                if l == 0:
                    sre = s0[hb][:, 0:128]
                    sim = s0[hb][:, 128:256]
                    nc.tensor.matmul(pgr[:], sre, mat(gR + 128, gR + 256),
                                     start=True, stop=False)
                    nc.tensor.matmul(pgr[:], sim, mat(gR, gR + 128),
                                     start=False, stop=True)
                    nc.tensor.matmul(pgi[:], sre, mat(gR + 256, gR + 384),
                                     start=True, stop=False)
                    nc.tensor.matmul(pgi[:], sim, mat(gR + 128, gR + 256),
                                     start=False, stop=True)
                else:
                    sv = sAB[hb][(l - 1) % 2]
                    nc.tensor.matmul(pgr[:], sv[:, 0:128],
                                     mat(gR + 128, gR + 256),
                                     start=True, stop=False)
                    nc.tensor.matmul(pgr[:], sv[:, 192:320],
                                     mat(gC + 128, gC + 256),
                                     start=False, stop=False)
                    nc.tensor.matmul(pgr[:], sv[:, 128:256],
                                     mat(gR, gR + 128),
                                     start=False, stop=False)
                    nc.tensor.matmul(pgr[:], sv[:, 320:448],
                                     mat(gC, gC + 128),
                                     start=False, stop=True)
                    nc.tensor.matmul(pgi[:], sv[:, 0:128],
                                     mat(gR + 256, gR + 384),
                                     start=True, stop=False)
                    nc.tensor.matmul(pgi[:], sv[:, 192:320],
                                     mat(gC + 256, gC + 384),
                                     start=False, stop=False)
                    nc.tensor.matmul(pgi[:], sv[:, 128:256],
                                     mat(gR + 128, gR + 256),
                                     start=False, stop=False)
                    nc.tensor.matmul(pgi[:], sv[:, 320:448],
                                     mat(gC + 128, gC + 256),
                                     start=False, stop=True)
                return pgr, pgi

            def emit_k(l, hb, kR, kB):
                t = tt[hb]
                if l < DEPTH - 1:
                    pkr = ppool.tile([128, 128], F32, tag="ps",
                                     name=f"pkr{l}{hb}")
                    pki = ppool.tile([128, 128], F32, tag="ps",
                                     name=f"pki{l}{hb}")
                    nc.tensor.matmul(pkr[:], t[:, 0:128],
                                     mat(kR + 128, kR + 256),
                                     start=True, stop=False)
                    nc.tensor.matmul(pkr[:], t[:, 192:320],
                                     mat(kB + 128, kB + 256),
                                     start=False, stop=False)
                    nc.tensor.matmul(pkr[:], t[:, 128:256],
                                     mat(kR, kR + 128),
                                     start=False, stop=False)
                    nc.tensor.matmul(pkr[:], t[:, 320:448],
                                     mat(kB, kB + 128),
                                     start=False, stop=True)
                    nc.tensor.matmul(pki[:], t[:, 0:128],
                                     mat(kR + 256, kR + 384),
                                     start=True, stop=False)
                    nc.tensor.matmul(pki[:], t[:, 192:320],
                                     mat(kB + 256, kB + 384),
                                     start=False, stop=False)
                    nc.tensor.matmul(pki[:], t[:, 128:256],
                                     mat(kR + 128, kR + 256),
                                     start=False, stop=False)
                    nc.tensor.matmul(pki[:], t[:, 320:448],
                                     mat(kB + 128, kB + 256),
                                     start=False, stop=True)
                    return pkr, pki
                p3r = bpool.tile([128, 128], F32, tag="pb",
                                 name=f"pk3r{hb}")
                p3i = bpool.tile([128, 128], F32, tag="pb",
                                 name=f"pk3i{hb}")
                nc.tensor.matmul(p3r[:], t[:, 0:128],
                                 mat(kR + 128, kR + 256),
                                 start=True, stop=False)
                nc.tensor.matmul(p3r[:], t[:, 192:320],
                                 mat(kB + 128, kB + 256),
                                 start=False, stop=False)
                nc.tensor.matmul(p3r[:], t[:, 128:256],
                                 mat(kR, kR + 128),
                                 start=False, stop=False)
                nc.tensor.matmul(p3r[:], t[:, 320:448],
                                 mat(kB, kB + 128),
                                 start=False, stop=True)
                nc.tensor.matmul(p3i[:], t[:, 0:128],
                                 mat(kR + 256, kR + 384),
                                 start=True, stop=False)
                nc.tensor.matmul(p3i[:], t[:, 192:320],
                                 mat(kB + 256, kB + 384),
                                 start=False, stop=False)
                nc.tensor.matmul(p3i[:], t[:, 128:256],
                                 mat(kR + 128, kR + 256),
                                 start=False, stop=False)
                nc.tensor.matmul(p3i[:], t[:, 320:448],
                                 mat(kB + 128, kB + 256),
                                 start=False, stop=True)
                psK3[hb] = (p3r, p3i)
                return None

            for l in range(DEPTH):
                base = L_OFF[l]
                gR = base
                gC = base + 384
                kR = base + (768 if l >= 1 else 384)
                kB = kR + 384
                pg = [emit_g(l, hb, gR, gC) for hb in range(2)]
                for hb in range(2):
                    tch = chunks(tt[hb])
                    nc.scalar.copy(
                        tch[:, :, 0],
                        pg[hb][0][:].rearrange("p (par c) -> p par c", par=2))
                    nc.vector.tensor_copy(
                        tch[:, :, 1],
                        pg[hb][1][:].rearrange("p (par c) -> p par c", par=2))
                pk = [emit_k(l, hb, kR, kB) for hb in range(2)]
                if l < DEPTH - 1:
                    for hb in range(2):
                        sch = chunks(sAB[hb][l % 2])
                        nc.scalar.copy(
                            sch[:, :, 0],
                            pk[hb][0][:].rearrange("p (w9 c) -> p w9 c",
                                                   w9=2))
                        nc.vector.tensor_copy(
                            sch[:, :, 1],
                            pk[hb][1][:].rearrange("p (w9 c) -> p w9 c",
                                                   w9=2))

            # ---------------- projection ----------------
            SQUARE = mybir.ActivationFunctionType.Square
            for hb in range(2):
                sq = cpool.tile([128, 32], F32, tag=f"sq{hb}", name=f"sq{hb}")
                rs = cpool.tile([128, 2], F32, tag=f"rs{hb}", name=f"rs{hb}")
                for ri in range(2):
                    nc.scalar.activation(
                        sq[:, ri * 16:ri * 16 + 16],
                        psK3[hb][ri][:, 0:16], SQUARE,
                        accum_out=rs[:, ri:ri + 1])
                psq = ppool.tile([16, 1], F32, tag="ps", name=f"q{hb}")
                nc.tensor.matmul(psq[:], t_pk32[:, 0:16], rs[:, 0:1],
                                 start=True, stop=False)
                nc.tensor.matmul(psq[:], t_pk32[:, 0:16], rs[:, 1:2],
                                 start=False, stop=False)
                nc.tensor.matmul(psq[:], t_pk32[0:1, 18:34],
                                 t_pk32[0:1, 16:17], start=False, stop=True)
                res = cpool.tile([16, 1], F32, tag=f"res{hb}",
                                 name=f"res{hb}")
                nc.vector.tensor_copy(res[:], psq[:])
                nc.sync.dma_start(d_out[:, hb * HB:hb * HB + HB], res[:])

    nc.compile()
    return nc


# ---------------------------------------------------------------------------
# Entry point
# ---------------------------------------------------------------------------


def kernel(features, weights):
    global _PROGRAM
    from concourse.bass_utils import run_bass_kernel_spmd

    features = np.asarray(features)
    weights = np.asarray(weights)
    if _PROGRAM is None:
        _PROGRAM = _build_program()
    nc = _PROGRAM

    mats = _make_mats(weights)
    pk32 = _make_pk32()
    in_maps = []
    for c in range(NCORES):
        fc = features[c * B_CORE:(c + 1) * B_CORE]
        in_maps.append({
            "pk16": _make_pk16(fc),
            "pk32": pk32,
            "mats": mats,
        })

    last_err = None
    for attempt in range(3):
        try:
            res = run_bass_kernel_spmd(nc, in_maps, list(range(NCORES)))
            break
        except Exception as e:  # noqa: BLE001
            last_err = e
            import time

            time.sleep(10 * (attempt + 1))
    else:
        raise last_err
    out = np.concatenate([res.results[c]["out"][0] for c in range(NCORES)])
    return out.astype(np.float32)


if __name__ == "__main__":
    rng = np.random.default_rng(0)
    f = rng.standard_normal((256, 10)).astype(np.float32)
    w = (0.01 * rng.random((4, 10))).astype(np.float32)
    print(kernel(f, w)[:8])


# revision 16
# speedup vs baseline: 1.0259x; 1.0259x over previous
"""Trainium2 Bass kernel for the 14-wire quantum autoencoder swap test.

Math: wires 10-13 stay |0> until the swap test, so
P(aux=1) = 0.5 - 0.5*q with q = sum_{trash wires 7,8,9 = 0} |c_i|^2 of the
10-qubit state after AngleEmbedding + BasicEntanglerLayers.

All transforms run on the PE in fp16 (fp32 PSUM accumulation):
  state S~ [ft, w] per 16-sample half-batch, ft = par*64 + w0*32 + w1*16 + bl
  (par = w2 after C01,C12; w = wires 3..9 index, w9 = MSB).
  Per layer: stage G (RX0-2 + C01 + C12 [+ C90 of previous layer folded via
  row-permuted GkC90 blocks for the w9=1 column half]) flips to standard
  layout [w, ft']; stage K (RX3-9 + C34..C89, C23 via K2/K2b stationary split
  by par class) flips back. Zero-padded stationary columns keep every matmul
  writing all 128 PSUM partitions (fp32-family dst-partition ISA rule).
  Negations live in host-built blocks: [Xim_neg | Xre | Xim] per matrix.
Final: |.|^2 on w<16 cols, sel-matmul partition reduce, affine.
"""
import numpy as np

NCORES = 8
B_CORE = 32
HB = 16
DEPTH = 4
NQ = 10

# pk16 fp16 [128, 1088]: fsc 512 | pt 384 ([imn|re|im]) | zeros 192
C_FSC = 0          # per half: [Fre 128 | Fim 128] at rows hb*32+bl
C_PT = 512
C_ZERO = 896
PK16 = 1088

# pk32 f32 [128, 34]: cols 0:16 = -0.5*sel, col 16 = 1.0, cols 18:34 = 0.5
PK32 = 34

# mats fp16 [128, 5760]: per layer: gR 384 | gC 384 (l>=1) | kR 384 | kB 384
L_COLS = [1152, 1536, 1536, 1536]
L_OFF = [0, 1152, 2688, 4224]
M_COLS = 5760

# ---------------------------------------------------------------------------
# Host-side constant construction
# ---------------------------------------------------------------------------

# ft class (par*4 + gg) -> g = w0*4 + w1*2 + w2
_FT_G = np.zeros(8, dtype=np.int64)
for _par in range(2):
    for _gg in range(4):
        _FT_G[_par * 4 + _gg] = (_gg >> 1) * 4 + (_gg & 1) * 2 + _par


def _perm_matrix(perm):
    m = np.zeros((len(perm), len(perm)))
    for src, dst in enumerate(perm):
        m[dst, src] = 1.0
    return m


def _cnot_chain_perm_p():
    perm = np.zeros(128, dtype=np.int64)
    for p in range(128):
        w = [(p >> k) & 1 for k in range(7)]
        for k in range(6):
            w[k + 1] ^= w[k]
        perm[p] = sum(w[k] << k for k in range(7))
    return perm


def _build_k2(weights_l):
    m = np.array([[1.0]], dtype=np.complex128)
    for w in (9, 8, 7, 6, 5, 4, 3):
        c, s = np.cos(weights_l[w] / 2), np.sin(weights_l[w] / 2)
        r = np.array([[c, -1j * s], [-1j * s, c]], dtype=np.complex128)
        m = np.kron(m, r)
    qa = _perm_matrix(_cnot_chain_perm_p())
    k2 = qa @ m
    k2b = k2 @ _perm_matrix(np.arange(128) ^ 1)
    return k2, k2b


def _build_gk(weights_l):
    m = np.array([[1.0]], dtype=np.complex128)
    for w in (0, 1, 2):
        c, s = np.cos(weights_l[w] / 2), np.sin(weights_l[w] / 2)
        r = np.array([[c, -1j * s], [-1j * s, c]], dtype=np.complex128)
        m = np.kron(m, r)
    perm = np.zeros(8, dtype=np.int64)
    for g in range(8):
        w0, w1, w2 = (g >> 2) & 1, (g >> 1) & 1, g & 1
        w1 ^= w0
        w2 ^= w1
        perm[g] = w0 * 4 + w1 * 2 + w2
    G = _perm_matrix(perm) @ m
    gk = np.zeros((128, 128), dtype=np.complex128)
    for ci in range(8):
        for co in range(8):
            v = G[_FT_G[co], _FT_G[ci]]
            if v != 0:
                for bl in range(HB):
                    gk[ci * 16 + bl, co * 16 + bl] = v
    return gk  # contraction: T[ft_out] = sum_ft_in S[ft_in] * gk[ft_in, ft_out]


def _blocks(m):
    """[im_neg | re | im] fp column triple for complex matrix m [128,128]."""
    return np.concatenate([-m.imag, m.real, m.imag], axis=1)


def _make_mats(weights):
    wt = weights.astype(np.float64).reshape(DEPTH, NQ)
    flip = np.arange(128) ^ 32
    cols = []
    for l in range(DEPTH):
        gk = _build_gk(wt[l])
        k2, k2b = _build_k2(wt[l])
        cols.append(_blocks(gk))
        if l >= 1:
            cols.append(_blocks(gk[flip, :]))
        cols.append(_blocks(k2.T))
        cols.append(_blocks(k2b.T))
    mats = np.concatenate(cols, axis=1)
    assert mats.shape == (128, M_COLS)
    return mats.astype(np.float16)


def _make_pk16(features_core):
    th = features_core.astype(np.float64)
    B = th.shape[0]
    c_emb, s_emb = np.cos(th / 2), np.sin(th / 2)
    v = np.stack([c_emb.astype(np.complex128), -1j * s_emb], axis=-1)

    pk = np.zeros((128, PK16), dtype=np.float64)
    # rows: sample b = hb*16+bl lives at partition hb*32+bl
    rows = np.array([(b // HB) * 32 + (b % HB) for b in range(B)])
    # pt: seed over wires 3..9: col j = w9*64+...+w3 (w3 = LSB)
    for j in range(128):
        val = np.ones(B, dtype=np.complex128)
        for k, w in enumerate((3, 4, 5, 6, 7, 8, 9)):
            val = val * v[:, w, (j >> k) & 1]
        pk[rows, C_PT + j] = -val.imag
        pk[rows, C_PT + 128 + j] = val.real
        pk[rows, C_PT + 256 + j] = val.imag
    # fsc: per half: Fre/Fim [16, 128]; col ft = cls*16+bl, row hb*32+bl
    for hb in range(2):
        for cls in range(8):
            g = _FT_G[cls]
            w0, w1, w2 = (g >> 2) & 1, (g >> 1) & 1, g & 1
            for bl in range(HB):
                b = hb * HB + bl
                F = v[b, 0, w0] * v[b, 1, w1] * v[b, 2, w2]
                col = C_FSC + hb * 256
                pk[hb * 32 + bl, col + cls * 16 + bl] = F.real
                pk[hb * 32 + bl, col + 128 + cls * 16 + bl] = F.imag
    return pk.astype(np.float16)


def _make_pk32():
    pk = np.zeros((128, PK32), dtype=np.float32)
    for ft in range(128):
        pk[ft, ft % 16] = -0.5
    pk[:, 16] = 1.0
    pk[:, 18:34] = 0.5
    return pk


# ---------------------------------------------------------------------------
# Bass program
# ---------------------------------------------------------------------------

_PROGRAM = None


def _build_program():
    import concourse.bacc as bacc
    import concourse.mybir as mybir
    import concourse.tile as tile

    F32 = mybir.dt.float32
    F16 = mybir.dt.float16
    MULT = mybir.AluOpType.mult
    ADD = mybir.AluOpType.add

    nc = bacc.Bacc("TRN2", target_bir_lowering=False, debug=False,
                   num_devices=NCORES)

    d_pk16 = nc.dram_tensor("pk16", [128, PK16], F16, kind="ExternalInput")
    d_pk32 = nc.dram_tensor("pk32", [128, PK32], F32, kind="ExternalInput")
    d_mats = nc.dram_tensor("mats", [128, M_COLS], F16, kind="ExternalInput")
    d_out = nc.dram_tensor("out", [1, B_CORE], F32, kind="ExternalOutput")

    with tile.TileContext(nc) as tc:
        with (
            tc.tile_pool(name="const", bufs=1) as cpool,
            tc.tile_pool(name="psum", bufs=6, space="PSUM") as ppool,
            tc.tile_pool(name="psumb", bufs=2, space="PSUM") as bpool,
        ):
            t_pk16 = cpool.tile([128, PK16], F16, tag="pk16")
            t_pk32 = cpool.tile([128, PK32], F32, tag="pk32")
            t_mats = cpool.tile([128, M_COLS], F16, tag="mats")

            # DMAs in just-in-time order, single SP queue
            nc.sync.dma_start(t_pk16[0:64, 0:C_ZERO], d_pk16[0:64, 0:C_ZERO])
            nc.sync.dma_start(t_mats[:, 0:384], d_mats[:, 0:384])
            nc.sync.dma_start(t_pk16[:, C_ZERO:PK16], d_pk16[:, C_ZERO:PK16])
            nc.sync.dma_start(t_mats[:, 384:1152], d_mats[:, 384:1152])
            for l in range(1, DEPTH):
                sl = slice(L_OFF[l], L_OFF[l] + L_COLS[l])
                nc.sync.dma_start(t_mats[:, sl], d_mats[:, sl])
            nc.sync.dma_start(t_pk32[:], d_pk32[:])

            # PE warm-up: starts the PE ramp clock early
            t_wu = cpool.tile([128, 256], F16, tag="wu")
            nc.gpsimd.memset(t_wu[:], 0.0)
            ps_wu = ppool.tile([128, 128], F32, tag="ps", name="wu")
            for i in range(3):
                nc.tensor.matmul(ps_wu[:], t_wu[:, 0:128], t_wu[:, 0:128],
                                 start=True, stop=True)

            # state tiles (persistent; zero-padded layout [128, 512]:
            # data chunks at 0,128,256,384 (64 cols), Z at 64,192,320)
            def zfill(t):
                dst = t[:].rearrange("p (a b) -> p a b", a=4, b=128)
                src = t_pk16[:, C_ZERO:C_ZERO + 192].rearrange(
                    "p (a b) -> p a b", a=3, b=64)
                nc.gpsimd.tensor_copy(dst[:, 0:3, 64:128], src)

            s0 = []
            sAB = []
            tt = []
            for hb in range(2):
                s0.append(cpool.tile([128, 256], F16, tag=f"s0{hb}",
                                     name=f"s0{hb}"))
                a = cpool.tile([128, 512], F16, tag=f"sA{hb}", name=f"sA{hb}")
                b = cpool.tile([128, 512], F16, tag=f"sB{hb}", name=f"sB{hb}")
                t = cpool.tile([128, 512], F16, tag=f"tt{hb}", name=f"tt{hb}")
                sAB.append([a, b])
                tt.append(t)
            for hb in range(2):
                zfill(tt[hb])
            for hb in range(2):
                zfill(sAB[hb][0])
            for hb in range(2):
                zfill(sAB[hb][1])

            # chunk views of a zero-padded tile: [p, x(par/w9), y(ri), 64]
            def chunks(t):
                return t[:].rearrange("p (x y b) -> p x y b",
                                      x=2, y=2, b=128)[:, :, :, 0:64]

            # embedding matmuls (re/im split) -> psE -> s0
            psE = []
            ev = []
            for hb in range(2):
                r0 = hb * 32
                rows = slice(r0, r0 + HB)
                fre = t_pk16[r0:r0 + HB, C_FSC + hb * 256:C_FSC + hb * 256 + 128]
                fim = t_pk16[r0:r0 + HB,
                             C_FSC + hb * 256 + 128:C_FSC + hb * 256 + 256]
                ptimn = t_pk16[rows, C_PT:C_PT + 128]
                ptre = t_pk16[rows, C_PT + 128:C_PT + 256]
                ptim = t_pk16[rows, C_PT + 256:C_PT + 384]
                ev.append((fre, fim, ptimn, ptre, ptim))
                psEr = ppool.tile([128, 128], F32, tag="ps", name=f"per{hb}")
                psEi = ppool.tile([128, 128], F32, tag="ps", name=f"pei{hb}")
                psE.append((psEr, psEi))
                nc.tensor.matmul(psEr[:], fre, ptre, start=True, stop=False)
                nc.tensor.matmul(psEr[:], fim, ptimn, start=False, stop=True)
            for hb in range(2):
                nc.scalar.copy(s0[hb][:, 0:128], psE[hb][0][:])
            for hb in range(2):
                fre, fim, ptimn, ptre, ptim = ev[hb]
                nc.tensor.matmul(psE[hb][1][:], fre, ptim,
                                 start=True, stop=False)
                nc.tensor.matmul(psE[hb][1][:], fim, ptre,
                                 start=False, stop=True)
            for hb in range(2):
                nc.vector.tensor_copy(s0[hb][:, 128:256], psE[hb][1][:])

            # ---------------- layers ----------------
            def mat(c0, c1):
                return t_mats[:, c0:c1]

            psK3 = [None, None]

            def emit_g(l, hb, gR, gC):
                pgr = ppool.tile([128, 128], F32, tag="ps",
                                 name=f"pgr{l}{hb}")
                pgi = ppool.tile([128, 128], F32, tag="ps",
                                 name=f"pgi{l}{hb}")
                if l == 0:
                    sre = s0[hb][:, 0:128]
                    sim = s0[hb][:, 128:256]
                    nc.tensor.matmul(pgr[:], sre, mat(gR + 128, gR + 256),
                                     start=True, stop=False)
                    nc.tensor.matmul(pgr[:], sim, mat(gR, gR + 128),
                                     start=False, stop=True)
                    nc.tensor.matmul(pgi[:], sre, mat(gR + 256, gR + 384),
                                     start=True, stop=False)
                    nc.tensor.matmul(pgi[:], sim, mat(gR + 128, gR + 256),
                                     start=False, stop=True)
                else:
                    sv = sAB[hb][(l - 1) % 2]
                    nc.tensor.matmul(pgr[:], sv[:, 0:128],
                                     mat(gR + 128, gR + 256),
                                     start=True, stop=False)
                    nc.tensor.matmul(pgr[:], sv[:, 192:320],
                                     mat(gC + 128, gC + 256),
                                     start=False, stop=False)
                    nc.tensor.matmul(pgr[:], sv[:, 128:256],
                                     mat(gR, gR + 128),
                                     start=False, stop=False)
                    nc.tensor.matmul(pgr[:], sv[:, 320:448],
                                     mat(gC, gC + 128),
                                     start=False, stop=True)
                    nc.tensor.matmul(pgi[:], sv[:, 0:128],
                                     mat(gR + 256, gR + 384),
                                     start=True, stop=False)
                    nc.tensor.matmul(pgi[:], sv[:, 192:320],
                                     mat(gC + 256, gC + 384),
                                     start=False, stop=False)
                    nc.tensor.matmul(pgi[:], sv[:, 128:256],
                                     mat(gR + 128, gR + 256),
                                     start=False, stop=False)
                    nc.tensor.matmul(pgi[:], sv[:, 320:448],
                                     mat(gC + 128, gC + 256),
                                     start=False, stop=True)
                return pgr, pgi

            def emit_k(l, hb, kR, kB):
                t = tt[hb]
                if l < DEPTH - 1:
                    pkr = ppool.tile([128, 128], F32, tag="ps",
                                     name=f"pkr{l}{hb}")
                    pki = ppool.tile([128, 128], F32, tag="ps",
                                     name=f"pki{l}{hb}")
                    nc.tensor.matmul(pkr[:], t[:, 0:128],
                                     mat(kR + 128, kR + 256),
                                     start=True, stop=False)
                    nc.tensor.matmul(pkr[:], t[:, 192:320],
                                     mat(kB + 128, kB + 256),
                                     start=False, stop=False)
                    nc.tensor.matmul(pkr[:], t[:, 128:256],
                                     mat(kR, kR + 128),
                                     start=False, stop=False)
                    nc.tensor.matmul(pkr[:], t[:, 320:448],
                                     mat(kB, kB + 128),
                                     start=False, stop=True)
                    nc.tensor.matmul(pki[:], t[:, 0:128],
                                     mat(kR + 256, kR + 384),
                                     start=True, stop=False)
                    nc.tensor.matmul(pki[:], t[:, 192:320],
                                     mat(kB + 256, kB + 384),
                                     start=False, stop=False)
                    nc.tensor.matmul(pki[:], t[:, 128:256],
                                     mat(kR + 128, kR + 256),
                                     start=False, stop=False)
                    nc.tensor.matmul(pki[:], t[:, 320:448],
                                     mat(kB + 128, kB + 256),
                                     start=False, stop=True)
                    return pkr, pki
                psK3[hb] = bpool.tile([128, 256], F32, tag="pb",
                                      name=f"pk3{hb}")
                nc.tensor.matmul(psK3[hb][:], t[:, 0:128],
                                 mat(kR + 128, kR + 384),
                                 start=True, stop=False)
                nc.tensor.matmul(psK3[hb][:], t[:, 192:320],
                                 mat(kB + 128, kB + 384),
                                 start=False, stop=False)
                nc.tensor.matmul(psK3[hb][:], t[:, 128:256],
                                 mat(kR, kR + 256),
                                 start=False, stop=False)
                nc.tensor.matmul(psK3[hb][:], t[:, 320:448],
                                 mat(kB, kB + 256),
                                 start=False, stop=True)
                return None

            for l in range(DEPTH):
                base = L_OFF[l]
                gR = base
                gC = base + 384
                kR = base + (768 if l >= 1 else 384)
                kB = kR + 384
                pg = [emit_g(l, hb, gR, gC) for hb in range(2)]
                for hb in range(2):
                    tch = chunks(tt[hb])
                    nc.scalar.copy(
                        tch[:, :, 0],
                        pg[hb][0][:].rearrange("p (par c) -> p par c", par=2))
                    nc.vector.tensor_copy(
                        tch[:, :, 1],
                        pg[hb][1][:].rearrange("p (par c) -> p par c", par=2))
                pk = [emit_k(l, hb, kR, kB) for hb in range(2)]
                if l < DEPTH - 1:
                    for hb in range(2):
                        sch = chunks(sAB[hb][l % 2])
                        nc.scalar.copy(
                            sch[:, :, 0],
                            pk[hb][0][:].rearrange("p (w9 c) -> p w9 c",
                                                   w9=2))
                        nc.vector.tensor_copy(
                            sch[:, :, 1],
                            pk[hb][1][:].rearrange("p (w9 c) -> p w9 c",
                                                   w9=2))

            # ---------------- projection ----------------
            SQUARE = mybir.ActivationFunctionType.Square
            for hb in range(2):
                sq = cpool.tile([128, 32], F32, tag=f"sq{hb}", name=f"sq{hb}")
                rs = cpool.tile([128, 1], F32, tag=f"rs{hb}", name=f"rs{hb}")
                v = psK3[hb][:].rearrange("p (ri c) -> p ri c",
                                          ri=2)[:, :, 0:16]
                nc.scalar.activation(
                    sq[:].rearrange("p (ri c) -> p ri c", ri=2), v, SQUARE,
                    accum_out=rs[:])
                psq = ppool.tile([16, 1], F32, tag="ps", name=f"q{hb}")
                nc.tensor.matmul(psq[:], t_pk32[:, 0:16], rs[:],
                                 start=True, stop=False)
                nc.tensor.matmul(psq[:], t_pk32[0:1, 18:34],
                                 t_pk32[0:1, 16:17], start=False, stop=True)
                res = cpool.tile([16, 1], F32, tag=f"res{hb}",
                                 name=f"res{hb}")
                nc.vector.tensor_copy(res[:], psq[:])
                nc.sync.dma_start(d_out[:, hb * HB:hb * HB + HB], res[:])

    nc.compile()
    return nc


# ---------------------------------------------------------------------------
# Entry point
# ---------------------------------------------------------------------------


def kernel(features, weights):
    global _PROGRAM
    from concourse.bass_utils import run_bass_kernel_spmd

    features = np.asarray(features)
    weights = np.asarray(weights)
    if _PROGRAM is None:
        _PROGRAM = _build_program()
    nc = _PROGRAM

    mats = _make_mats(weights)
    pk32 = _make_pk32()
    in_maps = []
    for c in range(NCORES):
        fc = features[c * B_CORE:(c + 1) * B_CORE]
        in_maps.append({
            "pk16": _make_pk16(fc),
            "pk32": pk32,
            "mats": mats,
        })

    last_err = None
    for attempt in range(3):
        try:
            res = run_bass_kernel_spmd(nc, in_maps, list(range(NCORES)))
            break
        except Exception as e:  # noqa: BLE001
            last_err = e
            import time

            time.sleep(10 * (attempt + 1))
    else:
        raise last_err
    out = np.concatenate([res.results[c]["out"][0] for c in range(NCORES)])
    return out.astype(np.float32)


if __name__ == "__main__":
    rng = np.random.default_rng(0)
    f = rng.standard_normal((256, 10)).astype(np.float32)
    w = (0.01 * rng.random((4, 10))).astype(np.float32)
    print(kernel(f, w)[:8])
